# revision 26
# baseline (speedup 1.0000x reference)
"""Multi-head self-attention (B=2, L=2048, D=1024, H=16, causal) on 8
Trainium2 NeuronCores.

Sharding: tensor-parallel over heads x data-parallel over batch.
Core c (0..7) handles batch b = c//4 and heads 4*(c%4) .. 4*(c%4)+3.
Each core computes partial = (softmax(qk^T/8) @ v_heads) @ Wo[:, cols]^T of
shape [L, D]; the host sums the 4 partials of each batch group.

Per-core kernel, v2 (all matmul operands bf16, PSUM accumulation fp32):
  - host supplies x^T (chunk-major packed) and pre-transposed bf16 weights;
    DMA lands directly in the matmul-input tiles -- no staging copies
  - q^T,k^T [256,L] come from PE matmuls c-chunk-outer so compute starts
    as soon as the first x^T chunk chunk lands; v [L,256] is computed
    per-L-tile with a memset ones column per head for free softmax
    denominators
  - scores are computed TRANSPOSED (S^T = k q^T per 128-row key chunk,
    causal tiles only), exp runs on ScalarE PSUM->SBUF producing P^T in
    bf16 exactly as the PV matmul consumes it; ScalarE runs nothing else
  - the softmax denominator row (from the v ones column) is inverted on
    VectorE, broadcast across partitions with a K=1 ones matmul (f32r
    bitcast), and the normalize multiply lands the attention output in a
    head-pair-stacked [128, L] layout
  - the output projection therefore contracts full 128-partition chunks
    (2 matmuls per 512 output cols) and is DMA'd to DRAM straight from
    PSUM
  - attention is software-pipelined one step ahead (emit S of step s+1
    before PV of step s) across head boundaries so the PE never waits on
    ScalarE exp latency
"""

import numpy as np

B, L, D, H = 2, 2048, 1024, 16
DH = D // H  # 64
N_CORES = 8
HEADS_PER_CORE = 4
HD = HEADS_PER_CORE * DH  # 256 head dims per core
NK = D // 128  # 8 contraction chunks
LT = L // 128  # 16 L tiles
NG = L // 512  # 4 column groups

_CACHE = {}


# ---------------------------------------------------------------------------
# walrus compat: this compiler build accepts at most ONE sync-wait command
# per instruction, while TileContext attaches one wait per producer proc.
# Hoist surplus waits onto same-engine NOPs inserted just before the
# offending instruction (identical AND semantics).
# ---------------------------------------------------------------------------
def _split_waits(nc):
    import bass_rust
    import concourse.mybir as mybir

    for fn in nc.m.functions:
        for bb in fn.blocks:
            insts = list(bb.instructions)
            out = []
            changed = False
            for inst in insts:
                si = inst.sync_info
                waits = list(si.on_wait) if si is not None and si.on_wait else []
                if len(waits) > 1:
                    changed = True
                    for w in waits[:-1]:
                        out.append(
                            mybir.InstNoOp(
                                name=nc.get_next_instruction_name(),
                                engine=inst.engine,
                                bass_nofuse=True,
                                sync_info=bass_rust.SyncInfo(
                                    on_wait=[w], on_update=[]
                                ),
                            )
                        )
                    inst.sync_info = bass_rust.SyncInfo(
                        on_wait=[waits[-1]], on_update=list(si.on_update or [])
                    )
                out.append(inst)
            if changed:
                try:
                    bb.instructions = out
                except Exception:
                    bb.instructions.clear()
                    bb.instructions.extend(out)


def _build_program():
    import concourse.bass as bass
    import concourse.mybir as mybir
    import concourse.tile as tile

    f32 = mybir.dt.float32
    f32r = mybir.dt.float32r
    bf16 = mybir.dt.bfloat16
    AF = mybir.ActivationFunctionType

    nc = bass.Bass("TRN2", target_bir_lowering=False, debug=False)
    # host-packed layouts (see _make_in_maps):
    #   xT    [128, NK*L]    bf16   [p, c*L + l] = x[l, c*128+p]
    #   wqkv  [128, NK*768]  bf16   [p, c*768 + s*256 + i] = Ws[c*128+p, i]
    #                               (s = 0/1/2 for q/k/v; Ws = W[sel,:].T)
    #   wo    [128, 2*D]     bf16   [p, j*D + i] = Wo[:, sel].T[j*128+p, i]
    #   trimask [128, 128]   bf16   0 lower-tri / -1e5 strictly-upper
    #   ident [128, 128]     bf16   identity (mask-accumulate matmul lhsT)
    xT_d = nc.dram_tensor("xT", [128, NK * L], bf16, kind="ExternalInput")
    wqkv_d = nc.dram_tensor("wqkv", [128, NK * 3 * HD], bf16, kind="ExternalInput")
    wo_d = nc.dram_tensor("wo", [128, 2 * D], bf16, kind="ExternalInput")
    tm_d = nc.dram_tensor("trimask", [128, 128], bf16, kind="ExternalInput")
    id_d = nc.dram_tensor("ident", [128, 128], bf16, kind="ExternalInput")
    out_d = nc.dram_tensor("out", [L, D], bf16, kind="ExternalOutput")

    with tile.TileContext(nc, pool_alloc_mode="queue") as tc:
        with tc.tile_pool(name="persist", bufs=1) as persist:
            xTr = persist.tile([128, NK, L], bf16)
            wqkv = persist.tile([128, NK, 3 * HD], bf16)
            woTr = persist.tile([128, 2, D], bf16)
            qTr = persist.tile([128, 2, L], bf16)
            kTr = persist.tile([128, 2, L], bf16)
            # per pair j: LT tiles of [65 x 2] (64 head dims + ones col)
            v_sb = persist.tile([128, 2, LT, 2 * (DH + 1)], bf16)
            tm_t = persist.tile([128, 128], bf16)
            ones_f = persist.tile([1, 64], f32r)
            ot = persist.tile([128, 2, L], bf16)

            id_t = persist.tile([128, 128], bf16)

            # interleave weight/x chunks so projection round c can start as
            # soon as its two chunks land; round-0 chunks lead everything
            for c in range(NK):
                nc.sync.dma_start(
                    wqkv[:, c, :], wqkv_d[:, c * 3 * HD : (c + 1) * 3 * HD]
                )
                if c == 0:
                    # chunk 0 lands in 512-col pieces so the first projection
                    # matmul starts ~2us earlier
                    for g in range(NG):
                        nc.sync.dma_start(
                            xTr[:, 0, g * 512 : (g + 1) * 512],
                            xT_d[:, g * 512 : (g + 1) * 512],
                        )
                else:
                    nc.sync.dma_start(xTr[:, c, :], xT_d[:, c * L : (c + 1) * L])
                if c == NK - 1:
                    # mask/identity are first needed by the first score chunk,
                    # well after the projection stream starts
                    nc.sync.dma_start(tm_t[:], tm_d[:])
                    nc.sync.dma_start(id_t[:], id_d[:])
            nc.sync.dma_start(woTr[:], wo_d[:].rearrange("p (j i) -> p j i", j=2))
            nc.vector.memset(ones_f[:], 1.0)
            # ones columns, one simple strided memset per (pair, head):
            # fancier multi-dim APs fail walrus ISA codegen
            for j in range(2):
                for hh in range(2):
                    nc.vector.memset(
                        v_sb[:, j, :, 65 * hh + DH : 65 * hh + DH + 1], 1.0
                    )

            # ---------------- phase A: pair-0 projections ----------------
            # q/k c-chunk outer so compute starts once chunk 0 lands; v after
            # (needs every chunk anyway).
            with tc.tile_pool(name="psA", bufs=1, space="PSUM") as psA:
                qps = [
                    psA.tile([128, 512], f32, name=f"qp0_{g}", tag=f"pA{2 * g}")
                    for g in range(NG)
                ]
                kps = [
                    psA.tile([128, 512], f32, name=f"kp0_{g}", tag=f"pA{2 * g + 1}")
                    for g in range(NG)
                ]
                for c in range(NK):
                    wq_c = wqkv[:, c, 0:128]
                    wk_c = wqkv[:, c, HD : HD + 128]
                    for g in range(NG):
                        nc.tensor.matmul(
                            qps[g][:], wq_c, xTr[:, c, g * 512 : (g + 1) * 512],
                            start=(c == 0), stop=(c == NK - 1),
                        )
                        nc.tensor.matmul(
                            kps[g][:], wk_c, xTr[:, c, g * 512 : (g + 1) * 512],
                            start=(c == 0), stop=(c == NK - 1),
                        )
                # evacuations split across VectorE/ScalarE so the first
                # score chunk is not serialized behind one engine
                for g in range(NG):
                    nc.vector.tensor_copy(qTr[:, 0, g * 512 : (g + 1) * 512], qps[g][:])
                    if g == 0:
                        nc.scalar.copy(kTr[:, 0, 0:512], kps[0][:])
                    else:
                        nc.vector.tensor_copy(
                            kTr[:, 0, g * 512 : (g + 1) * 512], kps[g][:]
                        )

            # ------- phase B: attention, phase-decoupled across heads -------
            # Head h's scores+exp stream (S-phase) runs while head h-1's
            # PV stream -- whose inputs were all produced a phase ago and
            # so never waits on exp -- plus filler units (pair-1
            # projections, output projection) keep the PE saturated.
            # Fillers are paced by an ACT-minus-PE credit estimate.
            with (
                tc.tile_pool(name="ptp", bufs=3) as ptp,
                tc.tile_pool(name="rsp", bufs=2) as rsp,
                tc.tile_pool(name="bcp", bufs=2) as bcp,
                tc.tile_pool(name="outst", bufs=3) as outst,
                tc.tile_pool(name="psST", bufs=2, space="PSUM") as psST,
                tc.tile_pool(name="psPV", bufs=1, space="PSUM") as psPV,
                tc.tile_pool(name="psF", bufs=1, space="PSUM") as psF,
            ):
                fidx = [0]

                def ftile(name):
                    i = fidx[0] = fidx[0] + 1
                    return psF.tile([128, 512], f32, name=f"{name}_{i}", tag=f"f{i % 2}")

                def mk_qk_unit(which, g):
                    def emit():
                        ps = ftile(f"{which}1_{g}")
                        w0 = (0 if which == "q" else HD) + 128
                        for c in range(NK):
                            nc.tensor.matmul(
                                ps[:], wqkv[:, c, w0 : w0 + 128],
                                xTr[:, c, g * 512 : (g + 1) * 512],
                                start=(c == 0), stop=(c == NK - 1),
                            )
                        dst = qTr if which == "q" else kTr
                        nc.vector.tensor_copy(dst[:, 1, g * 512 : (g + 1) * 512], ps[:])
                    return emit, 4096

                def mk_v_unit(j, t):
                    def emit():
                        ps = ftile(f"v{j}_{t}")
                        for c in range(NK):
                            nc.tensor.matmul(
                                ps[:, 0:128],
                                xTr[:, c, t * 128 : (t + 1) * 128],
                                wqkv[:, c, 2 * HD + 128 * j : 2 * HD + 128 * (j + 1)],
                                start=(c == 0), stop=(c == NK - 1),
                            )
                        nc.vector.tensor_copy(
                            v_sb[:, j, t, :].rearrange("p (h u) -> p h u", u=DH + 1)[
                                :, :, 0:DH
                            ],
                            ps[:, 0:128].rearrange("p (h u) -> p h u", u=DH),
                        )
                    return emit, 1024

                stages = {}

                def mk_c_unit(t, n2):
                    def emit():
                        ps = ftile(f"c{t}_{n2}")
                        for j in range(2):
                            nc.tensor.matmul(
                                ps[:],
                                ot[:, j, t * 128 : (t + 1) * 128],
                                woTr[:, j, n2 * 512 : (n2 + 1) * 512],
                                start=(j == 0), stop=(j == 1),
                            )
                        # evacuation alternates ScalarE/VectorE so the drain
                        # after the last attention group is not single-engine
                        if n2 == 0:
                            stages[t] = outst.tile(
                                [128, D], bf16, tag="ostage", name=f"ostage{t}"
                            )
                            nc.scalar.copy(stages[t][:, 0:512], ps[:])
                        else:
                            nc.vector.tensor_copy(stages[t][:, 512:D], ps[:])
                        if n2 == 1:
                            nc.sync.dma_start(
                                out_d[t * 128 : (t + 1) * 128, :], stages.pop(t)[:]
                            )
                    return emit, 1024

                filler = []  # list of (kind, emit_fn, pe_cols), consumed front-first
                for t in range(LT):
                    filler.append(("v0",) + mk_v_unit(0, t))
                for g in range(NG):
                    filler.append(("qk",) + mk_qk_unit("q", g))
                    filler.append(("qk",) + mk_qk_unit("k", g))
                for t in range(LT):
                    filler.append(("v1",) + mk_v_unit(1, t))
                n_emitted = {"v0": 0, "qk": 0, "v1": 0, "c": 0}
                releases = []  # (release_step, unit): C units on a lag
                cur_step = [0]
                credit = [0.0]  # ns of PE slack accumulated vs the ACT stream

                def pump_one():
                    kind, fn, cols = filler.pop(0)
                    fn()
                    n_emitted[kind] += 1
                    credit[0] -= cols * 0.4167

                def pump_credit():
                    while filler and credit[0] > 0.0:
                        pump_one()

                def pump(n):
                    for _ in range(n):
                        if not filler:
                            return
                        pump_one()

                def pump_while(cond):
                    while filler and cond():
                        pump_one()

                pvt = {}  # (h, g) -> psum tile
                pts = {}  # (h, m) -> PT tile

                def emit_S(h, m):
                    hp, ho = h // 2, 64 * (h % 2)
                    c0 = 128 * m
                    w = L - c0
                    PT = ptp.tile([128, w], bf16, tag=f"pt{m}", name=f"pt{h}_{m}")
                    pts[(h, m)] = PT
                    for sub in range((w + 1023) // 1024):
                        s0 = c0 + 1024 * sub
                        sw = min(1024, L - s0)
                        stp = psST.tile(
                            [128, 1024], f32, tag="st", name=f"st{h}_{m}_{sub}"
                        )
                        if sub == 0:
                            # causal diagonal: accumulate the mask into PSUM
                            # with an identity matmul -- keeps the whole score
                            # chain on PE (no cross-engine mask add)
                            nc.tensor.matmul(
                                stp[:, 0:128],
                                kTr[ho : ho + 64, hp, c0 : c0 + 128],
                                qTr[ho : ho + 64, hp, c0 : c0 + 128],
                                start=True, stop=False,
                            )
                            nc.tensor.matmul(
                                stp[:, 0:128], id_t[:], tm_t[:],
                                start=False, stop=True,
                            )
                            lo = 128
                        else:
                            lo = 0
                        for n0 in range(lo, sw, 512):
                            nw = min(512, sw - n0)
                            if n0 == lo and lo == 128:
                                nw = min(384, sw - n0)
                            nc.tensor.matmul(
                                stp[:, n0 : n0 + nw],
                                kTr[ho : ho + 64, hp, c0 : c0 + 128],
                                qTr[ho : ho + 64, hp, s0 + n0 : s0 + n0 + nw],
                                start=True, stop=True,
                            )
                        nc.scalar.activation(
                            PT[:, 1024 * sub : 1024 * sub + sw],
                            stp[:, 0:sw],
                            AF.Exp,
                            scale=0.125,
                        )
                    credit[0] += (0.833 * w + 185.0 * ((w + 1023) // 1024)) - 0.4167 * (
                        w + 128
                    )

                def emit_pv_mm(h, g, m):
                    hp, par = h // 2, h % 2
                    c0, gs = 128 * m, 512 * g
                    if m == 0:
                        pvt[(h, g)] = psPV.tile(
                            [65, 512], f32, name=f"pv_h{h}_{g}", tag=f"pv{g % 2}"
                        )
                    pv = pvt[(h, g)]
                    r0 = max(gs, c0)
                    nc.tensor.matmul(
                        pv[:, r0 - gs : 512],
                        v_sb[:, hp, m, 65 * par : 65 * par + 65],
                        pts[(h, m)][:, r0 - c0 : gs + 512 - c0],
                        start=(m == 0),
                        stop=(m == min(LT - 1, 4 * g + 3)),
                    )
                    credit[0] -= 0.4167 * (gs + 512 - r0)

                def emit_pv_done(h, g):
                    # group g done: 1/denom on VectorE, K=1 ones matmul
                    # broadcasts it (f32r bitcast keeps full rate), ScalarE
                    # stages it to SBUF (DVE cannot read two PSUM operands),
                    # and the multiply writes the pair-stacked ot.
                    hp, par = h // 2, h % 2
                    gs = 512 * g
                    pv = pvt.pop((h, g))
                    # rs_row is produced directly as f32r: the BIR verifier
                    # requires matmul operands to be rounded to f32r by their
                    # producer (a plain f32 bitcast is rejected)
                    rs_row = rsp.tile([1, 512], f32r, tag="rs")
                    with nc.allow_low_precision(reason="softmax denom reciprocal"):
                        nc.vector.reciprocal(rs_row[:], pv[64:65, :])
                    bc_ps = ftile(f"bc_h{h}_{g}")
                    nc.tensor.matmul(
                        bc_ps[0:64, :],
                        ones_f[:],
                        rs_row[:],
                        start=True,
                        stop=True,
                    )
                    bc_sb = bcp.tile([64, 512], f32, tag="bc", name=f"bcs{h}_{g}")
                    nc.scalar.copy(bc_sb[:], bc_ps[0:64, :])
                    nc.vector.tensor_mul(
                        ot[64 * par : 64 * par + 64, hp, gs : gs + 512],
                        pv[0:64, :],
                        bc_sb[:],
                    )
                    credit[0] -= 0.4167 * 512
                    if h == HEADS_PER_CORE - 1:
                        # all heads done for query group g: queue the output
                        # projection for its four L-tiles (released with a
                        # lag so the C matmuls never wait on this group's
                        # normalize chain)
                        for t in range(4 * g, 4 * g + 4):
                            for n2 in range(2):
                                releases.append(
                                    (cur_step[0] + 2, ("c",) + mk_c_unit(t, n2))
                                )

                def mk_pv_ops(h):
                    ops = []
                    for g in range(NG):
                        for m in range(min(LT, 4 * g + 4)):
                            ops.append((h, g, m))
                        ops.append((h, g, None))
                    return ops

                for p in range(HEADS_PER_CORE + 1):
                    if p == 2:
                        # heads 2/3 read pair-1 q/k: flush those units
                        pump_while(lambda: n_emitted["qk"] < 2 * NG)
                    pv_ops = mk_pv_ops(p - 1) if p >= 1 else []
                    nsteps = LT if p < HEADS_PER_CORE else max(1, len(pv_ops) // 4)
                    for i in range(nsteps):
                        cur_step[0] = i
                        # PV ops and fillers lead; the S chunk -- whose PSUM
                        # rotation may have to wait on the exp stream -- is
                        # emitted last so its stall overlaps this step's work
                        take = -(-len(pv_ops) // (nsteps - i))  # ceil
                        for h, g, m in pv_ops[:take]:
                            if m is None:
                                emit_pv_done(h, g)
                            else:
                                vk = "v0" if h < 2 else "v1"
                                pump_while(
                                    lambda: n_emitted[vk] <= min(m, LT - 1)
                                )
                                emit_pv_mm(h, g, m)
                        pv_ops = pv_ops[take:]
                        if p < HEADS_PER_CORE:
                            pump_credit()
                            emit_S(p, i)
                        else:
                            while releases and releases[0][0] <= i:
                                filler.append(releases.pop(0)[1])
                            # taper so enough ready units remain to cover the
                            # final group's normalize chain latency
                            pump(2 if i < 6 else 1)
                filler.extend(u for _, u in releases)
                releases.clear()
                pump_while(lambda: True)

    _split_waits(nc)
    return nc


def _build_runner(nc):
    """Build the sharded PJRT executable once (mirrors
    bass2jax.run_bass_via_pjrt) and return a callable in_maps -> results."""
    import jax
    import numpy as _np
    from jax.sharding import Mesh, PartitionSpec
    from jax.experimental.shard_map import shard_map
    from concourse import bass2jax, mybir

    bass2jax.install_neuronx_cc_hook()
    partition_name = (
        nc.partition_id_tensor.name if nc.partition_id_tensor else None
    )
    in_names, out_names, out_avals, zero_outs = [], [], [], []
    for alloc in nc.m.functions[0].allocations:
        if not isinstance(alloc, mybir.MemoryLocationSet):
            continue
        name = alloc.memorylocations[0].name
        if alloc.kind == "ExternalInput":
            if name != partition_name:
                in_names.append(name)
        elif alloc.kind == "ExternalOutput":
            out_names.append(name)
            shape = tuple(alloc.tensor_shape)
            dtype = mybir.dt.np(alloc.dtype)
            out_avals.append(jax.core.ShapedArray(shape, dtype))
            zero_outs.append(_np.zeros(shape, dtype))
    n_params = len(in_names)
    n_outs = len(out_names)
    all_in_names = list(in_names) + list(out_names)
    if partition_name is not None:
        all_in_names.append(partition_name)
    donate = tuple(range(n_params, n_params + n_outs))

    def _body(*args):
        operands = list(args)
        if partition_name is not None:
            operands.append(bass2jax.partition_id_tensor())
        outs = bass2jax._bass_exec_p.bind(
            *operands,
            out_avals=tuple(out_avals),
            in_names=tuple(all_in_names),
            out_names=tuple(out_names),
            lowering_input_output_aliases=(),
            sim_require_finite=True,
            sim_require_nnan=True,
            nc=nc,
        )
        return tuple(outs)

    devices = jax.devices()[:N_CORES]
    mesh = Mesh(_np.asarray(devices), ("core",))
    in_specs = (PartitionSpec("core"),) * (n_params + n_outs)
    out_specs = (PartitionSpec("core"),) * n_outs
    sharded = jax.jit(
        shard_map(
            _body, mesh=mesh, in_specs=in_specs, out_specs=out_specs,
            check_rep=False,
        ),
        donate_argnums=donate,
        keep_unused=True,
    )

    def run(in_maps):
        concat_in = [
            _np.concatenate([_np.asarray(m[nm]) for m in in_maps], axis=0)
            for nm in in_names
        ]
        concat_zeros = [
            _np.zeros((N_CORES * z.shape[0], *z.shape[1:]), z.dtype)
            for z in zero_outs
        ]
        out_arrs = sharded(*concat_in, *concat_zeros)
        return [
            {
                nm: _np.asarray(out_arrs[i]).reshape(
                    N_CORES, *out_avals[i].shape
                )[c]
                for i, nm in enumerate(out_names)
            }
            for c in range(N_CORES)
        ]

    return run


def _numpy_ref(x, attn_mask, Wq, Wk, Wv, Wo):
    xb, Lb, Db = x.shape
    dh = Db // H
    x64 = x.astype(np.float64)
    q = (x64 @ Wq.T.astype(np.float64)).reshape(xb, Lb, H, dh)
    k = (x64 @ Wk.T.astype(np.float64)).reshape(xb, Lb, H, dh)
    v = (x64 @ Wv.T.astype(np.float64)).reshape(xb, Lb, H, dh)
    scores = np.einsum("blhd,bmhd->bhlm", q, k) / np.sqrt(dh)
    scores = np.where(attn_mask[None, None, :, :] == 0, -np.inf, scores)
    scores -= scores.max(axis=-1, keepdims=True)
    e = np.exp(scores)
    attn = e / e.sum(axis=-1, keepdims=True)
    out = np.einsum("bhlm,bmhd->blhd", attn, v).reshape(xb, Lb, Db)
    return (out @ Wo.T.astype(np.float64)).astype(x.dtype)


def _trimask():
    j = np.arange(128)
    return np.where(j[None, :] >= j[:, None], 0.0, -1.0e5).astype(np.float32)


def _eye128():
    return np.eye(128, dtype=np.float32)


def _make_in_maps(x, Wq, Wk, Wv, Wo):
    import ml_dtypes

    bf16 = ml_dtypes.bfloat16
    tm = _trimask().astype(bf16)
    ident = _eye128().astype(bf16)
    # xT packed [128, NK*L]: [p, c*L + l] = x[b, l, c*128 + p]
    xTp = [
        np.ascontiguousarray(
            x[b].T.reshape(NK, 128, L).transpose(1, 0, 2).reshape(128, NK * L)
        ).astype(bf16)
        for b in range(B)
    ]
    in_maps = []
    for core in range(N_CORES):
        b = core // 4
        s0 = HD * (core % 4)
        sel = slice(s0, s0 + HD)
        # Ws = W[sel, :].T  -> [D, HD]; pack [p, c*768 + s*256 + i]
        ws = np.stack(
            [Wq[sel, :].T, Wk[sel, :].T, Wv[sel, :].T], axis=0
        )  # [3, D, HD]
        wqkv = np.ascontiguousarray(
            ws.reshape(3, NK, 128, HD).transpose(2, 1, 0, 3).reshape(128, NK * 3 * HD)
        ).astype(bf16)
        # wo packed [p, j*D + i] = Wo[:, sel].T[j*128+p, i]
        woT = Wo[:, sel].T  # [HD, D]
        wo = np.ascontiguousarray(
            woT.reshape(2, 128, D).transpose(1, 0, 2).reshape(128, 2 * D)
        ).astype(bf16)
        in_maps.append(
            {"xT": xTp[b], "wqkv": wqkv, "wo": wo, "trimask": tm, "ident": ident}
        )
    return in_maps


def kernel(x, attn_mask, Wq, Wk, Wv, Wo):
    x = np.asarray(x)
    attn_mask = np.asarray(attn_mask)
    Wq, Wk, Wv, Wo = (np.asarray(a) for a in (Wq, Wk, Wv, Wo))
    causal = x.shape == (B, L, D) and np.array_equal(
        attn_mask != 0, np.tril(np.ones((L, L), dtype=bool))
    )
    if not causal:
        return _numpy_ref(x, attn_mask, Wq, Wk, Wv, Wo)

    if "run" not in _CACHE:
        _CACHE["run"] = _build_runner(_build_program())
    in_maps = _make_in_maps(x, Wq, Wk, Wv, Wo)
    results = _CACHE["run"](in_maps)
    out = np.zeros((B, L, D), dtype=np.float32)
    for c in range(N_CORES):
        out[c // 4] += results[c]["out"].astype(np.float32)
    return out


# revision 35
# speedup vs baseline: 1.0478x; 1.0478x over previous
"""Multi-head self-attention (B=2, L=2048, D=1024, H=16, causal) on 8
Trainium2 NeuronCores.

Sharding: tensor-parallel over heads x data-parallel over batch.
Core c (0..7) handles batch b = c//4 and heads 4*(c%4) .. 4*(c%4)+3.
Each core computes partial = (softmax(qk^T/8) @ v_heads) @ Wo[:, cols]^T of
shape [L, D]; the host sums the 4 partials of each batch group.

Per-core kernel, v2 (all matmul operands bf16, PSUM accumulation fp32):
  - host supplies x^T (chunk-major packed) and pre-transposed bf16 weights;
    DMA lands directly in the matmul-input tiles -- no staging copies --
    and pair-0 q/k projections run contraction-chunk-outer so compute
    starts as soon as the first x^T chunk lands
  - scores are computed TRANSPOSED (S^T = k q^T per 128-row key chunk,
    causal tiles only); the causal mask of the diagonal block is
    accumulated into PSUM by an identity matmul (pure-PE chain); exp runs
    on ScalarE PSUM->SBUF producing P^T in bf16 exactly as the PV matmul
    consumes it, and ScalarE runs nothing else during attention
  - softmax denominators ride along as a ones column appended to v; the
    raw denominator row is broadcast across partitions with a K=1 ones
    matmul (f32r row), inverted on VectorE during the PSUM->SBUF
    evacuation, and the normalize multiply lands the attention output in
    a head-pair-stacked [128, L] layout so the output projection
    contracts full 128-partition chunks
  - scheduling is phase-decoupled: head h's S/exp stream overlaps head
    h-1's PV stream (whose inputs are a full phase old, so it never
    waits on exp), while pair-1 projections and the per-query-group
    output projection are pumped into the PE stream as filler units
    paced by an ACT-minus-PE credit estimate; per step the S chunk is
    emitted LAST so its PSUM-rotation wait overlaps the step's work
    (the PE executes strictly in order)
"""

import numpy as np

B, L, D, H = 2, 2048, 1024, 16
DH = D // H  # 64
N_CORES = 8
HEADS_PER_CORE = 4
HD = HEADS_PER_CORE * DH  # 256 head dims per core
NK = D // 128  # 8 contraction chunks
LT = L // 128  # 16 L tiles
NG = L // 512  # 4 column groups

_CACHE = {}


# ---------------------------------------------------------------------------
# walrus compat: this compiler build accepts at most ONE sync-wait command
# per instruction, while TileContext attaches one wait per producer proc.
# Hoist surplus waits onto same-engine NOPs inserted just before the
# offending instruction (identical AND semantics).
# ---------------------------------------------------------------------------
def _split_waits(nc):
    import bass_rust
    import concourse.mybir as mybir

    for fn in nc.m.functions:
        for bb in fn.blocks:
            insts = list(bb.instructions)
            out = []
            changed = False
            for inst in insts:
                si = inst.sync_info
                waits = list(si.on_wait) if si is not None and si.on_wait else []
                if len(waits) > 1:
                    changed = True
                    for w in waits[:-1]:
                        out.append(
                            mybir.InstNoOp(
                                name=nc.get_next_instruction_name(),
                                engine=inst.engine,
                                bass_nofuse=True,
                                sync_info=bass_rust.SyncInfo(
                                    on_wait=[w], on_update=[]
                                ),
                            )
                        )
                    inst.sync_info = bass_rust.SyncInfo(
                        on_wait=[waits[-1]], on_update=list(si.on_update or [])
                    )
                out.append(inst)
            if changed:
                try:
                    bb.instructions = out
                except Exception:
                    bb.instructions.clear()
                    bb.instructions.extend(out)


def _build_program():
    import concourse.bass as bass
    import concourse.mybir as mybir
    import concourse.tile as tile

    f32 = mybir.dt.float32
    f32r = mybir.dt.float32r
    bf16 = mybir.dt.bfloat16
    AF = mybir.ActivationFunctionType

    nc = bass.Bass("TRN2", target_bir_lowering=False, debug=False)
    # host-packed layouts (see _make_in_maps):
    #   xT    [128, NK*L]    bf16   [p, c*L + l] = x[l, c*128+p]
    #   wqkv  [128, NK*768]  bf16   [p, c*768 + s*256 + i] = Ws[c*128+p, i]
    #                               (s = 0/1/2 for q/k/v; Ws = W[sel,:].T)
    #   wo    [128, 2*D]     bf16   [p, j*D + i] = Wo[:, sel].T[j*128+p, i]
    #   trimask [128, 128]   bf16   0 lower-tri / -1e5 strictly-upper
    #   ident [128, 128]     bf16   identity (mask-accumulate matmul lhsT)
    xT_d = nc.dram_tensor("xT", [128, NK * L], bf16, kind="ExternalInput")
    wqkv_d = nc.dram_tensor("wqkv", [128, NK * 3 * HD], bf16, kind="ExternalInput")
    wo_d = nc.dram_tensor("wo", [128, 2 * D], bf16, kind="ExternalInput")
    tm_d = nc.dram_tensor("trimask", [128, 128], bf16, kind="ExternalInput")
    id_d = nc.dram_tensor("ident", [128, 128], bf16, kind="ExternalInput")
    ones_d = nc.dram_tensor("ones_r", [1, 64], f32r, kind="ExternalInput")
    out_d = nc.dram_tensor("out", [L, D], bf16, kind="ExternalOutput")

    with tile.TileContext(nc, pool_alloc_mode="queue") as tc:
        with tc.tile_pool(name="persist", bufs=1) as persist:
            xTr = persist.tile([128, NK, L], bf16)
            wqkv = persist.tile([128, NK, 3 * HD], bf16)
            woTr = persist.tile([128, 2, D], bf16)
            qTr = persist.tile([128, 2, L], bf16)
            kTr = persist.tile([128, 2, L], bf16)
            # per pair j: LT tiles of [65 x 2] (64 head dims + ones col)
            v_sb = persist.tile([128, 2, LT, 2 * (DH + 1)], bf16)
            tm_t = persist.tile([128, 128], bf16)
            ones_f = persist.tile([1, 64], f32r)
            ot = persist.tile([128, 2, L], bf16)

            id_t = persist.tile([128, 128], bf16)

            # interleave weight/x chunks so projection round c can start as
            # soon as its two chunks land; round-0 chunks lead everything
            for c in range(NK):
                nc.sync.dma_start(
                    wqkv[:, c, :], wqkv_d[:, c * 3 * HD : (c + 1) * 3 * HD]
                )
                if c == 0:
                    # chunk 0 lands in 512-col pieces so the first projection
                    # matmul starts ~2us earlier
                    for g in range(NG):
                        nc.sync.dma_start(
                            xTr[:, 0, g * 512 : (g + 1) * 512],
                            xT_d[:, g * 512 : (g + 1) * 512],
                        )
                else:
                    nc.sync.dma_start(xTr[:, c, :], xT_d[:, c * L : (c + 1) * L])
                if c == NK - 1:
                    # mask/identity are first needed by the first score chunk,
                    # well after the projection stream starts
                    nc.sync.dma_start(tm_t[:], tm_d[:])
                    nc.sync.dma_start(id_t[:], id_d[:])
                    nc.sync.dma_start(ones_f[:], ones_d[:])
            nc.sync.dma_start(woTr[:], wo_d[:].rearrange("p (j i) -> p j i", j=2))
            # ones columns: memset a small f32 scratch (f32 memset is the
            # only variant proven on hw; bf16/f32r memsets fail ISA codegen)
            # and cast-copy into each (pair, head) strided column
            ones_v = persist.tile([128, LT], f32)
            nc.vector.memset(ones_v[:], 1.0)
            for j in range(2):
                for hh in range(2):
                    nc.vector.tensor_copy(
                        v_sb[:, j, :, 65 * hh + DH : 65 * hh + DH + 1],
                        ones_v[:].rearrange("p (t u) -> p t u", u=1),
                    )

            # ---------------- phase A: pair-0 projections ----------------
            # q/k c-chunk outer so compute starts once chunk 0 lands; v after
            # (needs every chunk anyway).
            with tc.tile_pool(name="psA", bufs=1, space="PSUM") as psA:
                qps = [
                    psA.tile([128, 512], f32, name=f"qp0_{g}", tag=f"pA{2 * g}")
                    for g in range(NG)
                ]
                kps = [
                    psA.tile([128, 512], f32, name=f"kp0_{g}", tag=f"pA{2 * g + 1}")
                    for g in range(NG)
                ]
                for c in range(NK):
                    wq_c = wqkv[:, c, 0:128]
                    wk_c = wqkv[:, c, HD : HD + 128]
                    for g in range(NG):
                        nc.tensor.matmul(
                            qps[g][:], wq_c, xTr[:, c, g * 512 : (g + 1) * 512],
                            start=(c == 0), stop=(c == NK - 1),
                        )
                        if c == NK - 1:
                            # evacuate as soon as each group's accumulation
                            # closes, split across VectorE/ScalarE, so the
                            # first score chunk starts right after the round
                            nc.vector.tensor_copy(
                                qTr[:, 0, g * 512 : (g + 1) * 512], qps[g][:]
                            )
                        nc.tensor.matmul(
                            kps[g][:], wk_c, xTr[:, c, g * 512 : (g + 1) * 512],
                            start=(c == 0), stop=(c == NK - 1),
                        )
                        if c == NK - 1:
                            nc.scalar.copy(
                                kTr[:, 0, g * 512 : (g + 1) * 512], kps[g][:]
                            )

            # ------- phase B: attention, phase-decoupled across heads -------
            # Head h's scores+exp stream (S-phase) runs while head h-1's
            # PV stream -- whose inputs were all produced a phase ago and
            # so never waits on exp -- plus filler units (pair-1
            # projections, output projection) keep the PE saturated.
            # Fillers are paced by an ACT-minus-PE credit estimate.
            with (
                tc.tile_pool(name="ptp", bufs=3) as ptp,
                tc.tile_pool(name="rsp", bufs=2) as rsp,
                tc.tile_pool(name="bcp", bufs=2) as bcp,
                tc.tile_pool(name="outst", bufs=6) as outst,
                tc.tile_pool(name="psST", bufs=2, space="PSUM") as psST,
                tc.tile_pool(name="psPV", bufs=1, space="PSUM") as psPV,
                tc.tile_pool(name="psF", bufs=1, space="PSUM") as psF,
            ):
                fidx = [0]

                def ftile(name):
                    i = fidx[0] = fidx[0] + 1
                    return psF.tile([128, 512], f32, name=f"{name}_{i}", tag=f"f{i % 2}")

                def mk_qk_unit(which, g):
                    def emit():
                        ps = ftile(f"{which}1_{g}")
                        w0 = (0 if which == "q" else HD) + 128
                        for c in range(NK):
                            nc.tensor.matmul(
                                ps[:], wqkv[:, c, w0 : w0 + 128],
                                xTr[:, c, g * 512 : (g + 1) * 512],
                                start=(c == 0), stop=(c == NK - 1),
                            )
                        dst = qTr if which == "q" else kTr
                        nc.vector.tensor_copy(dst[:, 1, g * 512 : (g + 1) * 512], ps[:])
                    return emit, 4096

                def mk_v_unit(j, t):
                    def emit():
                        ps = ftile(f"v{j}_{t}")
                        for c in range(NK):
                            nc.tensor.matmul(
                                ps[:, 0:128],
                                xTr[:, c, t * 128 : (t + 1) * 128],
                                wqkv[:, c, 2 * HD + 128 * j : 2 * HD + 128 * (j + 1)],
                                start=(c == 0), stop=(c == NK - 1),
                            )
                        nc.vector.tensor_copy(
                            v_sb[:, j, t, :].rearrange("p (h u) -> p h u", u=DH + 1)[
                                :, :, 0:DH
                            ],
                            ps[:, 0:128].rearrange("p (h u) -> p h u", u=DH),
                        )
                    return emit, 1024

                stages = {}

                def mk_c_unit(t, n2):
                    def emit():
                        ps = ftile(f"c{t}_{n2}")
                        for j in range(2):
                            nc.tensor.matmul(
                                ps[:],
                                ot[:, j, t * 128 : (t + 1) * 128],
                                woTr[:, j, n2 * 512 : (n2 + 1) * 512],
                                start=(j == 0), stop=(j == 1),
                            )
                        # evacuation alternates ScalarE/VectorE so the drain
                        # after the last attention group is not single-engine
                        if n2 == 0:
                            stages[t] = outst.tile(
                                [128, D], bf16, tag="ostage", name=f"ostage{t}"
                            )
                            nc.scalar.copy(stages[t][:, 0:512], ps[:])
                        else:
                            nc.vector.tensor_copy(stages[t][:, 512:D], ps[:])
                        if n2 == 1:
                            nc.sync.dma_start(
                                out_d[t * 128 : (t + 1) * 128, :], stages.pop(t)[:]
                            )
                    return emit, 1024

                filler = []  # list of (kind, emit_fn, pe_cols), consumed front-first
                for t in range(LT):
                    filler.append(("v0",) + mk_v_unit(0, t))
                for g in range(NG):
                    filler.append(("qk",) + mk_qk_unit("q", g))
                    filler.append(("qk",) + mk_qk_unit("k", g))
                for t in range(LT):
                    filler.append(("v1",) + mk_v_unit(1, t))
                n_emitted = {"v0": 0, "qk": 0, "v1": 0, "c": 0}
                releases = []  # (release_step, unit): C units on a lag
                cur_step = [0]
                credit = [0.0]  # ns of PE slack accumulated vs the ACT stream

                def pump_one():
                    kind, fn, cols = filler.pop(0)
                    fn()
                    n_emitted[kind] += 1
                    credit[0] -= cols * 0.4167

                def pump_credit():
                    while filler and credit[0] > 0.0:
                        pump_one()

                def pump(n):
                    for _ in range(n):
                        if not filler:
                            return
                        pump_one()

                def pump_while(cond):
                    while filler and cond():
                        pump_one()

                pvt = {}  # (h, g) -> psum tile
                pts = {}  # (h, m) -> PT tile

                def emit_S(h, m):
                    hp, ho = h // 2, 64 * (h % 2)
                    c0 = 128 * m
                    w = L - c0
                    PT = ptp.tile([128, w], bf16, tag=f"pt{m}", name=f"pt{h}_{m}")
                    pts[(h, m)] = PT
                    for sub in range((w + 1023) // 1024):
                        s0 = c0 + 1024 * sub
                        sw = min(1024, L - s0)
                        stp = psST.tile(
                            [128, 1024], f32, tag="st", name=f"st{h}_{m}_{sub}"
                        )
                        if sub == 0:
                            # causal diagonal: accumulate the mask into PSUM
                            # with an identity matmul -- keeps the whole score
                            # chain on PE (no cross-engine mask add)
                            nc.tensor.matmul(
                                stp[:, 0:128],
                                kTr[ho : ho + 64, hp, c0 : c0 + 128],
                                qTr[ho : ho + 64, hp, c0 : c0 + 128],
                                start=True, stop=False,
                            )
                            nc.tensor.matmul(
                                stp[:, 0:128], id_t[:], tm_t[:],
                                start=False, stop=True,
                            )
                            segs = [(128, min(512, sw))] if sw > 128 else []
                            n0 = 512
                        else:
                            segs, n0 = [], 0
                        while n0 < sw:
                            segs.append((n0, min(n0 + 512, sw)))
                            n0 += 512
                        for a, b in segs:
                            nc.tensor.matmul(
                                stp[:, a:b],
                                kTr[ho : ho + 64, hp, c0 : c0 + 128],
                                qTr[ho : ho + 64, hp, s0 + a : s0 + b],
                                start=True, stop=True,
                            )
                        nc.scalar.activation(
                            PT[:, 1024 * sub : 1024 * sub + sw],
                            stp[:, 0:sw],
                            AF.Exp,
                            scale=0.125,
                        )
                    credit[0] += (0.833 * w + 185.0 * ((w + 1023) // 1024)) - 0.4167 * (
                        w + 128
                    )

                def emit_pv_mm(h, g, m):
                    hp, par = h // 2, h % 2
                    c0, gs = 128 * m, 512 * g
                    if m == 0:
                        pvt[(h, g)] = psPV.tile(
                            [65, 512], f32, name=f"pv_h{h}_{g}", tag=f"pv{g % 2}"
                        )
                    pv = pvt[(h, g)]
                    r0 = max(gs, c0)
                    nc.tensor.matmul(
                        pv[:, r0 - gs : 512],
                        v_sb[:, hp, m, 65 * par : 65 * par + 65],
                        pts[(h, m)][:, r0 - c0 : gs + 512 - c0],
                        start=(m == 0),
                        stop=(m == min(LT - 1, 4 * g + 3)),
                    )
                    credit[0] -= 0.4167 * (gs + 512 - r0)

                def emit_pv_done(h, g):
                    # group g done: 1/denom on VectorE, K=1 ones matmul
                    # broadcasts it (f32r bitcast keeps full rate), ScalarE
                    # stages it to SBUF (DVE cannot read two PSUM operands),
                    # and the multiply writes the pair-stacked ot.
                    hp, par = h // 2, h % 2
                    gs = 512 * g
                    pv = pvt.pop((h, g))
                    # broadcast the RAW denominator row (rs_row produced
                    # natively as f32r -- the BIR verifier rejects f32
                    # bitcasts into f32r matmuls), then take the reciprocal
                    # on VectorE while evacuating PSUM->SBUF. ScalarE stays
                    # exp-only, which is what paces the attention phases.
                    rs_row = rsp.tile([1, 512], f32r, tag="rs")
                    with nc.allow_low_precision(reason="f32r denom row"):
                        nc.vector.tensor_copy(rs_row[:], pv[64:65, :])
                    bc_ps = ftile(f"bc_h{h}_{g}")
                    nc.tensor.matmul(
                        bc_ps[0:64, :],
                        ones_f[:],
                        rs_row[:],
                        start=True,
                        stop=True,
                    )
                    bc_sb = bcp.tile([64, 512], f32, tag="bc", name=f"bcs{h}_{g}")
                    nc.vector.reciprocal(bc_sb[:], bc_ps[0:64, :])
                    nc.vector.tensor_mul(
                        ot[64 * par : 64 * par + 64, hp, gs : gs + 512],
                        pv[0:64, :],
                        bc_sb[:],
                    )
                    credit[0] -= 0.4167 * 512
                    if h == HEADS_PER_CORE - 1:
                        # all heads done for query group g: queue the output
                        # projection for its four L-tiles (released with a
                        # lag so the C matmuls never wait on this group's
                        # normalize chain)
                        for t in range(4 * g, 4 * g + 4):
                            for n2 in range(2):
                                releases.append(
                                    (cur_step[0] + 2, ("c",) + mk_c_unit(t, n2))
                                )

                def mk_pv_ops(h):
                    ops = []
                    for g in range(NG):
                        for m in range(min(LT, 4 * g + 4)):
                            ops.append((h, g, m))
                        ops.append((h, g, None))
                    return ops

                for p in range(HEADS_PER_CORE + 1):
                    if p == 2:
                        # heads 2/3 read pair-1 q/k: flush those units
                        pump_while(lambda: n_emitted["qk"] < 2 * NG)
                    pv_ops = mk_pv_ops(p - 1) if p >= 1 else []
                    nsteps = LT if p < HEADS_PER_CORE else max(1, len(pv_ops) // 4)
                    for i in range(nsteps):
                        cur_step[0] = i
                        # PV ops and fillers lead; the S chunk -- whose PSUM
                        # rotation may have to wait on the exp stream -- is
                        # emitted last so its stall overlaps this step's work
                        take = -(-len(pv_ops) // (nsteps - i))  # ceil
                        for h, g, m in pv_ops[:take]:
                            if m is None:
                                emit_pv_done(h, g)
                            else:
                                vk = "v0" if h < 2 else "v1"
                                pump_while(
                                    lambda: n_emitted[vk] <= min(m, LT - 1)
                                )
                                emit_pv_mm(h, g, m)
                        pv_ops = pv_ops[take:]
                        if p < HEADS_PER_CORE:
                            pump_credit()
                            emit_S(p, i)
                        else:
                            while releases and releases[0][0] <= i:
                                filler.append(releases.pop(0)[1])
                            pump(2)
                filler.extend(u for _, u in releases)
                releases.clear()
                pump_while(lambda: True)

    _split_waits(nc)
    return nc


def _build_runner(nc):
    """Build the sharded PJRT executable once (mirrors
    bass2jax.run_bass_via_pjrt) and return a callable in_maps -> results."""
    import jax
    import numpy as _np
    from jax.sharding import Mesh, PartitionSpec
    from jax.experimental.shard_map import shard_map
    from concourse import bass2jax, mybir

    bass2jax.install_neuronx_cc_hook()
    partition_name = (
        nc.partition_id_tensor.name if nc.partition_id_tensor else None
    )
    in_names, out_names, out_avals, zero_outs = [], [], [], []
    for alloc in nc.m.functions[0].allocations:
        if not isinstance(alloc, mybir.MemoryLocationSet):
            continue
        name = alloc.memorylocations[0].name
        if alloc.kind == "ExternalInput":
            if name != partition_name:
                in_names.append(name)
        elif alloc.kind == "ExternalOutput":
            out_names.append(name)
            shape = tuple(alloc.tensor_shape)
            dtype = mybir.dt.np(alloc.dtype)
            out_avals.append(jax.core.ShapedArray(shape, dtype))
            zero_outs.append(_np.zeros(shape, dtype))
    n_params = len(in_names)
    n_outs = len(out_names)
    all_in_names = list(in_names) + list(out_names)
    if partition_name is not None:
        all_in_names.append(partition_name)
    donate = tuple(range(n_params, n_params + n_outs))

    def _body(*args):
        operands = list(args)
        if partition_name is not None:
            operands.append(bass2jax.partition_id_tensor())
        outs = bass2jax._bass_exec_p.bind(
            *operands,
            out_avals=tuple(out_avals),
            in_names=tuple(all_in_names),
            out_names=tuple(out_names),
            lowering_input_output_aliases=(),
            sim_require_finite=True,
            sim_require_nnan=True,
            nc=nc,
        )
        return tuple(outs)

    devices = jax.devices()[:N_CORES]
    mesh = Mesh(_np.asarray(devices), ("core",))
    in_specs = (PartitionSpec("core"),) * (n_params + n_outs)
    out_specs = (PartitionSpec("core"),) * n_outs
    sharded = jax.jit(
        shard_map(
            _body, mesh=mesh, in_specs=in_specs, out_specs=out_specs,
            check_rep=False,
        ),
        donate_argnums=donate,
        keep_unused=True,
    )

    def run(in_maps):
        concat_in = [
            _np.concatenate([_np.asarray(m[nm]) for m in in_maps], axis=0)
            for nm in in_names
        ]
        concat_zeros = [
            _np.zeros((N_CORES * z.shape[0], *z.shape[1:]), z.dtype)
            for z in zero_outs
        ]
        out_arrs = sharded(*concat_in, *concat_zeros)
        return [
            {
                nm: _np.asarray(out_arrs[i]).reshape(
                    N_CORES, *out_avals[i].shape
                )[c]
                for i, nm in enumerate(out_names)
            }
            for c in range(N_CORES)
        ]

    return run


def _numpy_ref(x, attn_mask, Wq, Wk, Wv, Wo):
    xb, Lb, Db = x.shape
    dh = Db // H
    x64 = x.astype(np.float64)
    q = (x64 @ Wq.T.astype(np.float64)).reshape(xb, Lb, H, dh)
    k = (x64 @ Wk.T.astype(np.float64)).reshape(xb, Lb, H, dh)
    v = (x64 @ Wv.T.astype(np.float64)).reshape(xb, Lb, H, dh)
    scores = np.einsum("blhd,bmhd->bhlm", q, k) / np.sqrt(dh)
    scores = np.where(attn_mask[None, None, :, :] == 0, -np.inf, scores)
    scores -= scores.max(axis=-1, keepdims=True)
    e = np.exp(scores)
    attn = e / e.sum(axis=-1, keepdims=True)
    out = np.einsum("bhlm,bmhd->blhd", attn, v).reshape(xb, Lb, Db)
    return (out @ Wo.T.astype(np.float64)).astype(x.dtype)


def _trimask():
    j = np.arange(128)
    return np.where(j[None, :] >= j[:, None], 0.0, -1.0e5).astype(np.float32)


def _eye128():
    return np.eye(128, dtype=np.float32)


def _make_in_maps(x, Wq, Wk, Wv, Wo):
    import ml_dtypes

    bf16 = ml_dtypes.bfloat16
    tm = _trimask().astype(bf16)
    ident = _eye128().astype(bf16)
    # xT packed [128, NK*L]: [p, c*L + l] = x[b, l, c*128 + p]
    xTp = [
        np.ascontiguousarray(
            x[b].T.reshape(NK, 128, L).transpose(1, 0, 2).reshape(128, NK * L)
        ).astype(bf16)
        for b in range(B)
    ]
    in_maps = []
    for core in range(N_CORES):
        b = core // 4
        s0 = HD * (core % 4)
        sel = slice(s0, s0 + HD)
        # Ws = W[sel, :].T  -> [D, HD]; pack [p, c*768 + s*256 + i]
        ws = np.stack(
            [Wq[sel, :].T, Wk[sel, :].T, Wv[sel, :].T], axis=0
        )  # [3, D, HD]
        wqkv = np.ascontiguousarray(
            ws.reshape(3, NK, 128, HD).transpose(2, 1, 0, 3).reshape(128, NK * 3 * HD)
        ).astype(bf16)
        # wo packed [p, j*D + i] = Wo[:, sel].T[j*128+p, i]
        woT = Wo[:, sel].T  # [HD, D]
        wo = np.ascontiguousarray(
            woT.reshape(2, 128, D).transpose(1, 0, 2).reshape(128, 2 * D)
        ).astype(bf16)
        in_maps.append(
            {
                "xT": xTp[b], "wqkv": wqkv, "wo": wo, "trimask": tm,
                "ident": ident, "ones_r": np.ones((1, 64), dtype=np.float32),
            }
        )
    return in_maps


def kernel(x, attn_mask, Wq, Wk, Wv, Wo):
    x = np.asarray(x)
    attn_mask = np.asarray(attn_mask)
    Wq, Wk, Wv, Wo = (np.asarray(a) for a in (Wq, Wk, Wv, Wo))
    causal = x.shape == (B, L, D) and np.array_equal(
        attn_mask != 0, np.tril(np.ones((L, L), dtype=bool))
    )
    if not causal:
        return _numpy_ref(x, attn_mask, Wq, Wk, Wv, Wo)

    if "run" not in _CACHE:
        _CACHE["run"] = _build_runner(_build_program())
    in_maps = _make_in_maps(x, Wq, Wk, Wv, Wo)
    results = _CACHE["run"](in_maps)
    out = np.zeros((B, L, D), dtype=np.float32)
    for c in range(N_CORES):
        out[c // 4] += results[c]["out"].astype(np.float32)
    return out


# revision 36
# speedup vs baseline: 1.1424x; 1.0903x over previous
"""Multi-head self-attention (B=2, L=2048, D=1024, H=16, causal) on 8
Trainium2 NeuronCores.

Sharding: tensor-parallel over heads x data-parallel over batch.
Core c (0..7) handles batch b = c//4 and heads 4*(c%4) .. 4*(c%4)+3.
Each core computes partial = (softmax(qk^T/8) @ v_heads) @ Wo[:, cols]^T of
shape [L, D]; the host sums the 4 partials of each batch group.

Per-core kernel, v2 (all matmul operands bf16, PSUM accumulation fp32):
  - host supplies x^T (chunk-major packed) and pre-transposed bf16 weights;
    DMA lands directly in the matmul-input tiles -- no staging copies --
    and pair-0 q/k projections run contraction-chunk-outer so compute
    starts as soon as the first x^T chunk lands
  - scores are computed TRANSPOSED (S^T = k q^T per 128-row key chunk,
    causal tiles only); the causal mask of the diagonal block is
    accumulated into PSUM by an identity matmul (pure-PE chain); exp runs
    on ScalarE PSUM->SBUF producing P^T in bf16 exactly as the PV matmul
    consumes it, and ScalarE runs nothing else during attention
  - softmax denominators ride along as a ones column appended to v; the
    raw denominator row is broadcast across partitions with a K=1 ones
    matmul (f32r row), inverted on VectorE during the PSUM->SBUF
    evacuation, and the normalize multiply lands the attention output in
    a head-pair-stacked [128, L] layout so the output projection
    contracts full 128-partition chunks
  - scheduling is phase-decoupled: head h's S/exp stream overlaps head
    h-1's PV stream (whose inputs are a full phase old, so it never
    waits on exp), while pair-1 projections and the per-query-group
    output projection are pumped into the PE stream as filler units
    paced by an ACT-minus-PE credit estimate; per step the S chunk is
    emitted LAST so its PSUM-rotation wait overlaps the step's work
    (the PE executes strictly in order)
"""

import numpy as np

B, L, D, H = 2, 2048, 1024, 16
DH = D // H  # 64
N_CORES = 8
HEADS_PER_CORE = 4
HD = HEADS_PER_CORE * DH  # 256 head dims per core
NK = D // 128  # 8 contraction chunks
LT = L // 128  # 16 L tiles
NG = L // 512  # 4 column groups

_CACHE = {}


# ---------------------------------------------------------------------------
# walrus compat: this compiler build accepts at most ONE sync-wait command
# per instruction, while TileContext attaches one wait per producer proc.
# Hoist surplus waits onto same-engine NOPs inserted just before the
# offending instruction (identical AND semantics).
# ---------------------------------------------------------------------------
def _split_waits(nc):
    import bass_rust
    import concourse.mybir as mybir

    for fn in nc.m.functions:
        for bb in fn.blocks:
            insts = list(bb.instructions)
            out = []
            changed = False
            for inst in insts:
                si = inst.sync_info
                waits = list(si.on_wait) if si is not None and si.on_wait else []
                if len(waits) > 1:
                    changed = True
                    for w in waits[:-1]:
                        out.append(
                            mybir.InstNoOp(
                                name=nc.get_next_instruction_name(),
                                engine=inst.engine,
                                bass_nofuse=True,
                                sync_info=bass_rust.SyncInfo(
                                    on_wait=[w], on_update=[]
                                ),
                            )
                        )
                    inst.sync_info = bass_rust.SyncInfo(
                        on_wait=[waits[-1]], on_update=list(si.on_update or [])
                    )
                out.append(inst)
            if changed:
                try:
                    bb.instructions = out
                except Exception:
                    bb.instructions.clear()
                    bb.instructions.extend(out)


def _build_program():
    import concourse.bass as bass
    import concourse.mybir as mybir
    import concourse.tile as tile

    f32 = mybir.dt.float32
    f32r = mybir.dt.float32r
    bf16 = mybir.dt.bfloat16
    AF = mybir.ActivationFunctionType

    nc = bass.Bass("TRN2", target_bir_lowering=False, debug=False)
    # host-packed layouts (see _make_in_maps):
    #   xT    [128, NK*L]    bf16   [p, c*L + l] = x[l, c*128+p]
    #   wqkv  [128, NK*768]  bf16   [p, c*768 + s*256 + i] = Ws[c*128+p, i]
    #                               (s = 0/1/2 for q/k/v; Ws = W[sel,:].T)
    #   wo    [128, 2*D]     bf16   [p, j*D + i] = Wo[:, sel].T[j*128+p, i]
    #   trimask [128, 128]   bf16   0 lower-tri / -1e5 strictly-upper
    #   ident [128, 128]     bf16   identity (mask-accumulate matmul lhsT)
    xT_d = nc.dram_tensor("xT", [128, NK * L], bf16, kind="ExternalInput")
    wqkv_d = nc.dram_tensor("wqkv", [128, NK * 3 * HD], bf16, kind="ExternalInput")
    wo_d = nc.dram_tensor("wo", [128, 2 * D], bf16, kind="ExternalInput")
    tm_d = nc.dram_tensor("trimask", [128, 128], bf16, kind="ExternalInput")
    id_d = nc.dram_tensor("ident", [128, 128], bf16, kind="ExternalInput")
    ones_d = nc.dram_tensor("ones_r", [1, 64], f32r, kind="ExternalInput")
    out_d = nc.dram_tensor("out", [L, D], bf16, kind="ExternalOutput")

    with tile.TileContext(nc, pool_alloc_mode="queue") as tc:
        with tc.tile_pool(name="persist", bufs=1) as persist:
            xTr = persist.tile([128, NK, L], bf16)
            wqkv = persist.tile([128, NK, 3 * HD], bf16)
            woTr = persist.tile([128, 2, D], bf16)
            qTr = persist.tile([128, 2, L], bf16)
            kTr = persist.tile([128, 2, L], bf16)
            # per pair j: LT tiles of [65 x 2] (64 head dims + ones col)
            v_sb = persist.tile([128, 2, LT, 2 * (DH + 1)], bf16)
            tm_t = persist.tile([128, 128], bf16)
            ones_f = persist.tile([1, 64], f32r)
            ot = persist.tile([128, 2, L], bf16)

            id_t = persist.tile([128, 128], bf16)

            # interleave weight/x chunks so projection round c can start as
            # soon as its two chunks land; round-0 chunks lead everything
            for c in range(NK):
                nc.sync.dma_start(
                    wqkv[:, c, :], wqkv_d[:, c * 3 * HD : (c + 1) * 3 * HD]
                )
                if c == 0:
                    # chunk 0 lands in 512-col pieces so the first projection
                    # matmul starts ~2us earlier
                    for g in range(NG):
                        nc.sync.dma_start(
                            xTr[:, 0, g * 512 : (g + 1) * 512],
                            xT_d[:, g * 512 : (g + 1) * 512],
                        )
                else:
                    nc.sync.dma_start(xTr[:, c, :], xT_d[:, c * L : (c + 1) * L])
                if c == NK - 1:
                    # mask/identity are first needed by the first score chunk,
                    # well after the projection stream starts
                    nc.sync.dma_start(tm_t[:], tm_d[:])
                    nc.sync.dma_start(id_t[:], id_d[:])
                    nc.sync.dma_start(ones_f[:], ones_d[:])
            nc.sync.dma_start(woTr[:], wo_d[:].rearrange("p (j i) -> p j i", j=2))
            # ones columns: memset a small f32 scratch (f32 memset is the
            # only variant proven on hw; bf16/f32r memsets fail ISA codegen)
            # and cast-copy into each (pair, head) strided column
            ones_v = persist.tile([128, LT], f32)
            nc.vector.memset(ones_v[:], 1.0)
            for j in range(2):
                for hh in range(2):
                    nc.vector.tensor_copy(
                        v_sb[:, j, :, 65 * hh + DH : 65 * hh + DH + 1],
                        ones_v[:].rearrange("p (t u) -> p t u", u=1),
                    )

            # ---------------- phase A: pair-0 projections ----------------
            # q/k c-chunk outer so compute starts once chunk 0 lands; v after
            # (needs every chunk anyway).
            with tc.tile_pool(name="psA", bufs=1, space="PSUM") as psA:
                qps = [
                    psA.tile([128, 512], f32, name=f"qp0_{g}", tag=f"pA{2 * g}")
                    for g in range(NG)
                ]
                kps = [
                    psA.tile([128, 512], f32, name=f"kp0_{g}", tag=f"pA{2 * g + 1}")
                    for g in range(NG)
                ]
                for c in range(NK):
                    wq_c = wqkv[:, c, 0:128]
                    wk_c = wqkv[:, c, HD : HD + 128]
                    for g in range(NG):
                        nc.tensor.matmul(
                            qps[g][:], wq_c, xTr[:, c, g * 512 : (g + 1) * 512],
                            start=(c == 0), stop=(c == NK - 1),
                        )
                        if c == NK - 1:
                            # evacuate as soon as each group's accumulation
                            # closes, split across VectorE/ScalarE, so the
                            # first score chunk starts right after the round
                            nc.vector.tensor_copy(
                                qTr[:, 0, g * 512 : (g + 1) * 512], qps[g][:]
                            )
                        nc.tensor.matmul(
                            kps[g][:], wk_c, xTr[:, c, g * 512 : (g + 1) * 512],
                            start=(c == 0), stop=(c == NK - 1),
                        )
                        if c == NK - 1:
                            nc.scalar.copy(
                                kTr[:, 0, g * 512 : (g + 1) * 512], kps[g][:]
                            )

            # ------- phase B: attention, phase-decoupled across heads -------
            # Head h's scores+exp stream (S-phase) runs while head h-1's
            # PV stream -- whose inputs were all produced a phase ago and
            # so never waits on exp -- plus filler units (pair-1
            # projections, output projection) keep the PE saturated.
            # Fillers are paced by an ACT-minus-PE credit estimate.
            with (
                tc.tile_pool(name="ptp", bufs=3) as ptp,
                tc.tile_pool(name="rsp", bufs=2) as rsp,
                tc.tile_pool(name="bcp", bufs=2) as bcp,
                tc.tile_pool(name="outst", bufs=6) as outst,
                tc.tile_pool(name="psST", bufs=2, space="PSUM") as psST,
                tc.tile_pool(name="psPV", bufs=1, space="PSUM") as psPV,
                tc.tile_pool(name="psF", bufs=1, space="PSUM") as psF,
            ):
                fidx = [0]
                in_tail = [False]  # phase 4: the score PSUM banks are idle

                def ftile(name):
                    i = fidx[0] = fidx[0] + 1
                    if in_tail[0] and i % 3 == 2:
                        return psST.tile(
                            [128, 512], f32, name=f"{name}_{i}", tag="st"
                        )
                    return psF.tile([128, 512], f32, name=f"{name}_{i}", tag=f"f{i % 2}")

                def mk_qk_unit(which, g):
                    def emit():
                        ps = ftile(f"{which}1_{g}")
                        w0 = (0 if which == "q" else HD) + 128
                        for c in range(NK):
                            nc.tensor.matmul(
                                ps[:], wqkv[:, c, w0 : w0 + 128],
                                xTr[:, c, g * 512 : (g + 1) * 512],
                                start=(c == 0), stop=(c == NK - 1),
                            )
                        dst = qTr if which == "q" else kTr
                        nc.vector.tensor_copy(dst[:, 1, g * 512 : (g + 1) * 512], ps[:])
                    return emit, 4096

                def mk_v_unit(j, t):
                    def emit():
                        ps = ftile(f"v{j}_{t}")
                        for c in range(NK):
                            nc.tensor.matmul(
                                ps[:, 0:128],
                                xTr[:, c, t * 128 : (t + 1) * 128],
                                wqkv[:, c, 2 * HD + 128 * j : 2 * HD + 128 * (j + 1)],
                                start=(c == 0), stop=(c == NK - 1),
                            )
                        nc.vector.tensor_copy(
                            v_sb[:, j, t, :].rearrange("p (h u) -> p h u", u=DH + 1)[
                                :, :, 0:DH
                            ],
                            ps[:, 0:128].rearrange("p (h u) -> p h u", u=DH),
                        )
                    return emit, 1024

                stages = {}

                def mk_c_unit(t, n2):
                    def emit():
                        ps = ftile(f"c{t}_{n2}")
                        for j in range(2):
                            nc.tensor.matmul(
                                ps[:],
                                ot[:, j, t * 128 : (t + 1) * 128],
                                woTr[:, j, n2 * 512 : (n2 + 1) * 512],
                                start=(j == 0), stop=(j == 1),
                            )
                        # evacuation alternates ScalarE/VectorE so the drain
                        # after the last attention group is not single-engine
                        if n2 == 0:
                            stages[t] = outst.tile(
                                [128, D], bf16, tag="ostage", name=f"ostage{t}"
                            )
                            nc.scalar.copy(stages[t][:, 0:512], ps[:])
                        else:
                            nc.vector.tensor_copy(stages[t][:, 512:D], ps[:])
                        if n2 == 1:
                            nc.sync.dma_start(
                                out_d[t * 128 : (t + 1) * 128, :], stages.pop(t)[:]
                            )
                    return emit, 1024

                filler = []  # list of (kind, emit_fn, pe_cols), consumed front-first
                for t in range(LT):
                    filler.append(("v0",) + mk_v_unit(0, t))
                for g in range(NG):
                    filler.append(("qk",) + mk_qk_unit("q", g))
                    filler.append(("qk",) + mk_qk_unit("k", g))
                for t in range(LT):
                    filler.append(("v1",) + mk_v_unit(1, t))
                n_emitted = {"v0": 0, "qk": 0, "v1": 0, "c": 0}
                releases = []  # (release_step, unit): C units on a lag
                cur_step = [0]
                credit = [0.0]  # ns of PE slack accumulated vs the ACT stream

                def pump_one():
                    kind, fn, cols = filler.pop(0)
                    fn()
                    n_emitted[kind] += 1
                    credit[0] -= cols * 0.4167

                def pump_credit():
                    while filler and credit[0] > 0.0:
                        pump_one()

                def pump(n):
                    for _ in range(n):
                        if not filler:
                            return
                        pump_one()

                def pump_while(cond):
                    while filler and cond():
                        pump_one()

                pvt = {}  # (h, g) -> psum tile
                pts = {}  # (h, m) -> PT tile

                def emit_S(h, m):
                    hp, ho = h // 2, 64 * (h % 2)
                    c0 = 128 * m
                    w = L - c0
                    PT = ptp.tile([128, w], bf16, tag=f"pt{m}", name=f"pt{h}_{m}")
                    pts[(h, m)] = PT
                    for sub in range((w + 1023) // 1024):
                        s0 = c0 + 1024 * sub
                        sw = min(1024, L - s0)
                        stp = psST.tile(
                            [128, 1024], f32, tag="st", name=f"st{h}_{m}_{sub}"
                        )
                        if sub == 0:
                            # causal diagonal: accumulate the mask into PSUM
                            # with an identity matmul -- keeps the whole score
                            # chain on PE (no cross-engine mask add)
                            nc.tensor.matmul(
                                stp[:, 0:128],
                                kTr[ho : ho + 64, hp, c0 : c0 + 128],
                                qTr[ho : ho + 64, hp, c0 : c0 + 128],
                                start=True, stop=False,
                            )
                            nc.tensor.matmul(
                                stp[:, 0:128], id_t[:], tm_t[:],
                                start=False, stop=True,
                            )
                            segs = [(128, min(512, sw))] if sw > 128 else []
                            n0 = 512
                        else:
                            segs, n0 = [], 0
                        while n0 < sw:
                            segs.append((n0, min(n0 + 512, sw)))
                            n0 += 512
                        for a, b in segs:
                            nc.tensor.matmul(
                                stp[:, a:b],
                                kTr[ho : ho + 64, hp, c0 : c0 + 128],
                                qTr[ho : ho + 64, hp, s0 + a : s0 + b],
                                start=True, stop=True,
                            )
                        nc.scalar.activation(
                            PT[:, 1024 * sub : 1024 * sub + sw],
                            stp[:, 0:sw],
                            AF.Exp,
                            scale=0.125,
                        )
                    credit[0] += (0.833 * w + 185.0 * ((w + 1023) // 1024)) - 0.4167 * (
                        w + 128
                    )

                def emit_pv_mm(h, g, m):
                    hp, par = h // 2, h % 2
                    c0, gs = 128 * m, 512 * g
                    if m == 0:
                        pvt[(h, g)] = psPV.tile(
                            [65, 512], f32, name=f"pv_h{h}_{g}", tag=f"pv{g % 2}"
                        )
                    pv = pvt[(h, g)]
                    r0 = max(gs, c0)
                    nc.tensor.matmul(
                        pv[:, r0 - gs : 512],
                        v_sb[:, hp, m, 65 * par : 65 * par + 65],
                        pts[(h, m)][:, r0 - c0 : gs + 512 - c0],
                        start=(m == 0),
                        stop=(m == min(LT - 1, 4 * g + 3)),
                    )
                    credit[0] -= 0.4167 * (gs + 512 - r0)

                def emit_pv_done(h, g):
                    # group g done: 1/denom on VectorE, K=1 ones matmul
                    # broadcasts it (f32r bitcast keeps full rate), ScalarE
                    # stages it to SBUF (DVE cannot read two PSUM operands),
                    # and the multiply writes the pair-stacked ot.
                    hp, par = h // 2, h % 2
                    gs = 512 * g
                    pv = pvt.pop((h, g))
                    # broadcast the RAW denominator row (rs_row produced
                    # natively as f32r -- the BIR verifier rejects f32
                    # bitcasts into f32r matmuls), then take the reciprocal
                    # on VectorE while evacuating PSUM->SBUF. ScalarE stays
                    # exp-only, which is what paces the attention phases.
                    rs_row = rsp.tile([1, 512], f32r, tag="rs")
                    with nc.allow_low_precision(reason="f32r denom row"):
                        nc.vector.tensor_copy(rs_row[:], pv[64:65, :])
                    bc_ps = ftile(f"bc_h{h}_{g}")
                    nc.tensor.matmul(
                        bc_ps[0:64, :],
                        ones_f[:],
                        rs_row[:],
                        start=True,
                        stop=True,
                    )
                    bc_sb = bcp.tile([64, 512], f32, tag="bc", name=f"bcs{h}_{g}")
                    nc.vector.reciprocal(bc_sb[:], bc_ps[0:64, :])
                    nc.vector.tensor_mul(
                        ot[64 * par : 64 * par + 64, hp, gs : gs + 512],
                        pv[0:64, :],
                        bc_sb[:],
                    )
                    credit[0] -= 0.4167 * 512
                    if h == HEADS_PER_CORE - 1:
                        # all heads done for query group g: queue the output
                        # projection for its four L-tiles (released with a
                        # lag so the C matmuls never wait on this group's
                        # normalize chain)
                        for t in range(4 * g, 4 * g + 4):
                            for n2 in range(2):
                                releases.append(
                                    (cur_step[0] + 2, ("c",) + mk_c_unit(t, n2))
                                )

                def mk_pv_ops(h):
                    ops = []
                    for g in range(NG):
                        for m in range(min(LT, 4 * g + 4)):
                            ops.append((h, g, m))
                        ops.append((h, g, None))
                    return ops

                def emit_op(h, g, m, i):
                    if m is None:
                        emit_pv_done(h, g)
                    else:
                        vk = "v0" if h < 2 else "v1"
                        pump_while(lambda: n_emitted[vk] <= min(m, LT - 1))
                        emit_pv_mm(h, g, m)

                heads_ops = [mk_pv_ops(h) for h in range(HEADS_PER_CORE)]
                gstep = [0]  # global step clock for C releases
                for p in range(HEADS_PER_CORE + 1):
                    if p == 2:
                        # heads 2/3 read pair-1 q/k: flush those units
                        pump_while(lambda: n_emitted["qk"] < 2 * NG)
                    pv_ops = heads_ops[p - 1] if p >= 1 else []
                    in_tail[0] = p == HEADS_PER_CORE
                    nsteps = LT if p < HEADS_PER_CORE else max(1, len(pv_ops) // 4)
                    # front-load: finish this head's PV (and its normalize
                    # chains) a few steps before the phase ends, so the pv
                    # banks are free when the next phase needs them
                    fsteps = nsteps - 3 if p < HEADS_PER_CORE else nsteps
                    for i in range(nsteps):
                        gstep[0] += 1
                        cur_step[0] = gstep[0]
                        # PV ops and fillers lead; the S chunk -- whose PSUM
                        # rotation may have to wait on the exp stream -- is
                        # emitted last so its stall overlaps this step's work
                        if i < fsteps:
                            take = -(-len(pv_ops) // (fsteps - i))  # ceil
                        else:
                            take = len(pv_ops)
                        for op in pv_ops[:take]:
                            emit_op(*op, i)
                        pv_ops = pv_ops[take:]
                        heads_ops[p - 1 if p >= 1 else 0] = pv_ops
                        if p < HEADS_PER_CORE:
                            # tail of the phase: pull the next head's first
                            # query group forward into the exp shadow (its
                            # PT chunks and pv bank are ready by now)
                            if p >= 1 and not pv_ops and i >= 14:
                                nxt = heads_ops[p]
                                while nxt and (
                                    nxt[0][1] == 0
                                    and (nxt[0][2] is None or nxt[0][2] <= i - 2)
                                ):
                                    emit_op(*nxt.pop(0), i)
                            while releases and releases[0][0] <= gstep[0]:
                                filler.append(releases.pop(0)[1])
                            pump_credit()
                            emit_S(p, i)
                        else:
                            while releases and releases[0][0] <= gstep[0]:
                                filler.append(releases.pop(0)[1])
                            pump(2)
                filler.extend(u for _, u in releases)
                releases.clear()
                pump_while(lambda: True)

    _split_waits(nc)
    return nc


def _build_runner(nc):
    """Build the sharded PJRT executable once (mirrors
    bass2jax.run_bass_via_pjrt) and return a callable in_maps -> results."""
    import jax
    import numpy as _np
    from jax.sharding import Mesh, PartitionSpec
    from jax.experimental.shard_map import shard_map
    from concourse import bass2jax, mybir

    bass2jax.install_neuronx_cc_hook()
    partition_name = (
        nc.partition_id_tensor.name if nc.partition_id_tensor else None
    )
    in_names, out_names, out_avals, zero_outs = [], [], [], []
    for alloc in nc.m.functions[0].allocations:
        if not isinstance(alloc, mybir.MemoryLocationSet):
            continue
        name = alloc.memorylocations[0].name
        if alloc.kind == "ExternalInput":
            if name != partition_name:
                in_names.append(name)
        elif alloc.kind == "ExternalOutput":
            out_names.append(name)
            shape = tuple(alloc.tensor_shape)
            dtype = mybir.dt.np(alloc.dtype)
            out_avals.append(jax.core.ShapedArray(shape, dtype))
            zero_outs.append(_np.zeros(shape, dtype))
    n_params = len(in_names)
    n_outs = len(out_names)
    all_in_names = list(in_names) + list(out_names)
    if partition_name is not None:
        all_in_names.append(partition_name)
    donate = tuple(range(n_params, n_params + n_outs))

    def _body(*args):
        operands = list(args)
        if partition_name is not None:
            operands.append(bass2jax.partition_id_tensor())
        outs = bass2jax._bass_exec_p.bind(
            *operands,
            out_avals=tuple(out_avals),
            in_names=tuple(all_in_names),
            out_names=tuple(out_names),
            lowering_input_output_aliases=(),
            sim_require_finite=True,
            sim_require_nnan=True,
            nc=nc,
        )
        return tuple(outs)

    devices = jax.devices()[:N_CORES]
    mesh = Mesh(_np.asarray(devices), ("core",))
    in_specs = (PartitionSpec("core"),) * (n_params + n_outs)
    out_specs = (PartitionSpec("core"),) * n_outs
    sharded = jax.jit(
        shard_map(
            _body, mesh=mesh, in_specs=in_specs, out_specs=out_specs,
            check_rep=False,
        ),
        donate_argnums=donate,
        keep_unused=True,
    )

    def run(in_maps):
        concat_in = [
            _np.concatenate([_np.asarray(m[nm]) for m in in_maps], axis=0)
            for nm in in_names
        ]
        concat_zeros = [
            _np.zeros((N_CORES * z.shape[0], *z.shape[1:]), z.dtype)
            for z in zero_outs
        ]
        out_arrs = sharded(*concat_in, *concat_zeros)
        return [
            {
                nm: _np.asarray(out_arrs[i]).reshape(
                    N_CORES, *out_avals[i].shape
                )[c]
                for i, nm in enumerate(out_names)
            }
            for c in range(N_CORES)
        ]

    return run


def _numpy_ref(x, attn_mask, Wq, Wk, Wv, Wo):
    xb, Lb, Db = x.shape
    dh = Db // H
    x64 = x.astype(np.float64)
    q = (x64 @ Wq.T.astype(np.float64)).reshape(xb, Lb, H, dh)
    k = (x64 @ Wk.T.astype(np.float64)).reshape(xb, Lb, H, dh)
    v = (x64 @ Wv.T.astype(np.float64)).reshape(xb, Lb, H, dh)
    scores = np.einsum("blhd,bmhd->bhlm", q, k) / np.sqrt(dh)
    scores = np.where(attn_mask[None, None, :, :] == 0, -np.inf, scores)
    scores -= scores.max(axis=-1, keepdims=True)
    e = np.exp(scores)
    attn = e / e.sum(axis=-1, keepdims=True)
    out = np.einsum("bhlm,bmhd->blhd", attn, v).reshape(xb, Lb, Db)
    return (out @ Wo.T.astype(np.float64)).astype(x.dtype)


def _trimask():
    j = np.arange(128)
    return np.where(j[None, :] >= j[:, None], 0.0, -1.0e5).astype(np.float32)


def _eye128():
    return np.eye(128, dtype=np.float32)


def _make_in_maps(x, Wq, Wk, Wv, Wo):
    import ml_dtypes

    bf16 = ml_dtypes.bfloat16
    tm = _trimask().astype(bf16)
    ident = _eye128().astype(bf16)
    # xT packed [128, NK*L]: [p, c*L + l] = x[b, l, c*128 + p]
    xTp = [
        np.ascontiguousarray(
            x[b].T.reshape(NK, 128, L).transpose(1, 0, 2).reshape(128, NK * L)
        ).astype(bf16)
        for b in range(B)
    ]
    in_maps = []
    for core in range(N_CORES):
        b = core // 4
        s0 = HD * (core % 4)
        sel = slice(s0, s0 + HD)
        # Ws = W[sel, :].T  -> [D, HD]; pack [p, c*768 + s*256 + i]
        ws = np.stack(
            [Wq[sel, :].T, Wk[sel, :].T, Wv[sel, :].T], axis=0
        )  # [3, D, HD]
        wqkv = np.ascontiguousarray(
            ws.reshape(3, NK, 128, HD).transpose(2, 1, 0, 3).reshape(128, NK * 3 * HD)
        ).astype(bf16)
        # wo packed [p, j*D + i] = Wo[:, sel].T[j*128+p, i]
        woT = Wo[:, sel].T  # [HD, D]
        wo = np.ascontiguousarray(
            woT.reshape(2, 128, D).transpose(1, 0, 2).reshape(128, 2 * D)
        ).astype(bf16)
        in_maps.append(
            {
                "xT": xTp[b], "wqkv": wqkv, "wo": wo, "trimask": tm,
                "ident": ident, "ones_r": np.ones((1, 64), dtype=np.float32),
            }
        )
    return in_maps


def kernel(x, attn_mask, Wq, Wk, Wv, Wo):
    x = np.asarray(x)
    attn_mask = np.asarray(attn_mask)
    Wq, Wk, Wv, Wo = (np.asarray(a) for a in (Wq, Wk, Wv, Wo))
    causal = x.shape == (B, L, D) and np.array_equal(
        attn_mask != 0, np.tril(np.ones((L, L), dtype=bool))
    )
    if not causal:
        return _numpy_ref(x, attn_mask, Wq, Wk, Wv, Wo)

    if "run" not in _CACHE:
        _CACHE["run"] = _build_runner(_build_program())
    in_maps = _make_in_maps(x, Wq, Wk, Wv, Wo)
    results = _CACHE["run"](in_maps)
    out = np.zeros((B, L, D), dtype=np.float32)
    for c in range(N_CORES):
        out[c // 4] += results[c]["out"].astype(np.float32)
    return out
